# revision 7
# baseline (speedup 1.0000x reference)
"""Distributed Bass kernel for nn_CSNMModule_38663295598699 (sparse_attention).

Sharding: pure data parallel over B across the 8 trn2 NeuronCores — one
sample per core, params replicated.

Device kernel (per core, Tile framework, bf16/fp8):
  - embeds arrive in "X layout" [C=512, G=4096] fp8e4m3 — this is a pure
    view of the flat [N, D] buffer (torch-style layout-mixing reshape),
    and is simultaneously the avg_pool3d input layout (row = channel,
    free dims = 3D grid) and, viewed as [C, 8, 512], a token-grouped
    layout (row c holds tokens 8c..8c+7).
  - separable 3x3x3 sum-pool (the /27 is folded into W1 host-side) on
    the vector engine -> pooled P[s] in fp8, X layout.
  - per direction (6 of them): H^T = relu(W1k_top^T @ P_i + W1k_bot^T
    @ P_j + b1) via PE matmuls (K=channel on partitions), logits =
    w2^T @ H^T (M=1 matmuls), z = exp(logit + b2) on ACT with fused
    denominator accumulation, then numer = sum_n z[n] * e_j[n, :] via
    PE with z scattered to partitions (DRAM bounce) against the fp8 X
    tiles.
Host: exact fp32 token sums, fused mean, LayerNorm, final Wf matmul.

Self-contained: shapes hardcoded (B=8, N=4096, D=512, grids prod 4096).
"""

import numpy as np
import ml_dtypes

import concourse.bass as bass
import concourse.tile as tile
import concourse.mybir as mybir
from concourse.bass_utils import run_bass_kernel_spmd

# ---------------------------------------------------------------------------
# Workaround for this walrus build's per-instruction sync-wait limit: the
# TileContext kernel-tail drain piles every proc's sem wait onto one Drain
# instruction and CoreV3Gen rejects it ("Too many sync wait commands").
# Split the waits across one sync-engine nop each instead.
# ---------------------------------------------------------------------------
from concourse.vector_clock import ScopedClock


def _patched_drain_and_barrier(self, tick_clock, wait_clock):
    nc = self.nc
    probe = nc.sync.nop()
    wait_clock.add_sem_waits(probe.ins, ScopedClock({None: tick_clock.global_clock}))
    waits = list(probe.ins.sync_info.on_wait or [])
    probe.ins.sync_info.on_wait = waits[:1]
    for w in waits[1:]:
        n = nc.sync.nop()
        if n.ins.sync_info is None:
            n.ins.sync_info = mybir.SyncInfo(on_update=[], on_wait=[])
        n.ins.sync_info.on_wait.append(w)
    nc.sync.drain()
    nc.all_engine_barrier()
    popped = nc._tile_sem_poison_stack.pop()
    assert popped is self._sem_poison
    nc.clear_and_free_semaphores(list(self.sems.allocated().values()))
    nc.all_engine_barrier()


tile.TileContext._drain_and_barrier = _patched_drain_and_barrier

# Same walrus limit bites regular instructions once the kernel is large
# enough (e.g. a DMACopy that accumulates several producer waits).  Before
# lowering, split every instruction's waits so each instruction carries at
# most one, hoisting the rest onto same-engine nops placed just before it
# (program order on the engine makes this equivalent).
_orig_lower_ordered = tile.TileContext._lower_ordered_insts
_MAX_WAITS = 1


def _split_excess_waits_then_lower(self, ordered):
    nc = self.nc
    n_split = 0
    for insts in ordered.values():
        new = []
        for inst in insts:
            si = inst.sync_info
            waits = list(si.on_wait) if si is not None and si.on_wait else []
            if len(waits) > _MAX_WAITS:
                excess, keep = waits[:-_MAX_WAITS], waits[-_MAX_WAITS:]
                for w in excess:
                    nop = mybir.InstNoOp(
                        name=f"{inst.name}-wsplit{n_split}", ins=[], outs=[]
                    )
                    n_split += 1
                    nop.engine = inst.engine
                    nop.sync_info = mybir.SyncInfo(on_update=[], on_wait=[w])
                    nc.register_instruction(nop, overwrite=True)
                    new.append(nop)
                inst.sync_info = mybir.SyncInfo(
                    on_update=list(si.on_update) if si.on_update else [],
                    on_wait=keep,
                )
            new.append(inst)
        insts[:] = new
    return _orig_lower_ordered(self, ordered)


tile.TileContext._lower_ordered_insts = _split_excess_waits_then_lower

# ---------------------------------------------------------------------------

B, N, D = 8, 4096, 512
N_CORES = 8
GRIDS = ((16, 16, 16), (32, 16, 8), (8, 32, 16))  # prod == 4096
# (k, wi, wj, ej): mlp k applied to concat(windows[wi], windows[wj]),
# weighted sum over embeds[ej].  Order follows reference PAIRS.
DIRS = [(0, 0, 1, 1), (0, 1, 0, 0), (1, 0, 2, 2), (1, 2, 0, 0), (2, 1, 2, 2), (2, 2, 1, 1)]
N_ROWS = 3 * N + 6  # 12294

FP8 = mybir.dt.float8e4
BF16 = mybir.dt.bfloat16
F32 = mybir.dt.float32
NP_FP8 = ml_dtypes.float8_e4m3
NP_BF16 = ml_dtypes.bfloat16


def build_nc():
    nc = bass.Bass()
    xd = [nc.declare_dram_parameter(f"x{s}", [512, 4096], FP8, isOutput=False) for s in range(3)]
    w1d = nc.declare_dram_parameter("w1", [3, 1024, 512], BF16, isOutput=False)
    w2d = nc.declare_dram_parameter("w2", [3, 4, 128], BF16, isOutput=False)
    b1d = nc.declare_dram_parameter("b1", [3, 4, 128], F32, isOutput=False)
    b2d = nc.declare_dram_parameter("b2", [1, 3], F32, isOutput=False)
    onum = nc.declare_dram_parameter("onum", [6, 512], F32, isOutput=True)
    oden = nc.declare_dram_parameter("oden", [1, 6], F32, isOutput=True)

    Relu = mybir.ActivationFunctionType.Relu
    Exp = mybir.ActivationFunctionType.Exp

    with tile.TileContext(nc) as tc:
        with (
            tc.tile_pool(name="params", bufs=1) as params,
            tc.tile_pool(name="pres", bufs=1) as pres,         # pooled P, persistent
            tc.tile_pool(name="xs", bufs=3) as xpool,          # streamed X tiles
            tc.tile_pool(name="xn", bufs=5) as xnpool,         # streamed X tiles (numer)
            tc.tile_pool(name="pt", bufs=4) as tpool,          # pooling temps
            tc.tile_pool(name="hb", bufs=8) as hpool,          # relu'd H^T tiles
            tc.tile_pool(name="zb", bufs=2) as zpool,          # z vectors
            tc.tile_pool(name="zs", bufs=2) as zspool,         # scattered z
            tc.tile_pool(name="zd", bufs=2, space="DRAM") as zdram_pool,
            tc.tile_pool(name="hp", bufs=5, space="PSUM") as hpsum,
            tc.tile_pool(name="lp", bufs=2, space="PSUM") as lpsum,
            tc.tile_pool(name="np", bufs=1, space="PSUM") as npsum,
        ):
            # ---- params ----
            w1sb = [[params.tile([128, 512], BF16, tag=f"w1_{k}_{t}", name=f"w1sb_{k}_{t}") for t in range(8)]
                    for k in range(3)]
            for k in range(3):
                for t in range(8):
                    nc.sync.dma_start(w1sb[k][t][:], w1d[k, t * 128:(t + 1) * 128, :])
            w2sb = [params.tile([128, 4], BF16, tag=f"w2_{k}", name=f"w2sb_{k}") for k in range(3)]
            b1sb = [params.tile([128, 4], F32, tag=f"b1_{k}", name=f"b1sb_{k}") for k in range(3)]
            for k in range(3):
                nc.sync.dma_start(w2sb[k][:], w2d[k].rearrange("t p -> p t"))
                nc.sync.dma_start(b1sb[k][:], b1d[k].rearrange("t p -> p t"))
            b2sb = params.tile([1, 3], F32, tag="b2")
            nc.sync.dma_start(b2sb[:], b2d[:])
            denacc = params.tile([1, 48], F32, tag="denacc")
            den6 = params.tile([1, 6], F32, tag="den6")

            # ---- pooling: X[s] -> P[s] (fp8, X layout) ----
            P = [[None] * 4 for _ in range(3)]
            for s in range(3):
                d, h, w = GRIDS[s]
                for c in range(4):
                    xt = xpool.tile([128, 4096], FP8, tag="xs", name="xt")
                    nc.sync.dma_start(xt[:], xd[s][c * 128:(c + 1) * 128, :])
                    X4 = xt[:].rearrange("p (d h w) -> p d h w", d=d, h=h, w=w)

                    def v4(t):
                        return t[:].rearrange("p (d h w) -> p d h w", d=d, h=h, w=w)

                    # w axis: X -> bt
                    at = tpool.tile([128, 4096], BF16, tag="pt", name="ptile")
                    A = v4(at)
                    nc.vector.tensor_add(A[:, :, :, 1:], X4[:, :, :, 1:], X4[:, :, :, :w - 1])
                    nc.vector.tensor_copy(A[:, :, :, 0:1], X4[:, :, :, 0:1])
                    bt = tpool.tile([128, 4096], BF16, tag="pt", name="ptile")
                    Bv = v4(bt)
                    nc.vector.tensor_add(Bv[:, :, :, :w - 1], A[:, :, :, :w - 1], X4[:, :, :, 1:])
                    nc.vector.tensor_copy(Bv[:, :, :, w - 1:w], A[:, :, :, w - 1:w])
                    # h axis: bt -> dt
                    ct = tpool.tile([128, 4096], BF16, tag="pt", name="ptile")
                    C = v4(ct)
                    nc.vector.tensor_add(C[:, :, 1:, :], Bv[:, :, 1:, :], Bv[:, :, :h - 1, :])
                    nc.vector.tensor_copy(C[:, :, 0:1, :], Bv[:, :, 0:1, :])
                    dt = tpool.tile([128, 4096], BF16, tag="pt", name="ptile")
                    Dv = v4(dt)
                    nc.vector.tensor_add(Dv[:, :, :h - 1, :], C[:, :, :h - 1, :], Bv[:, :, 1:, :])
                    nc.vector.tensor_copy(Dv[:, :, h - 1:h, :], C[:, :, h - 1:h, :])
                    # d axis: dt -> P (fp8)
                    et = tpool.tile([128, 4096], BF16, tag="pt", name="ptile")
                    E3 = et[:].rearrange("p (d hw) -> p d hw", d=d)
                    D3 = dt[:].rearrange("p (d hw) -> p d hw", d=d)
                    nc.vector.tensor_add(E3[:, 1:, :], D3[:, 1:, :], D3[:, :d - 1, :])
                    nc.vector.tensor_copy(E3[:, 0:1, :], D3[:, 0:1, :])
                    pt_ = pres.tile([128, 4096], FP8, tag=f"P_{s}_{c}", name=f"P_{s}_{c}")
                    P3 = pt_[:].rearrange("p (d hw) -> p d hw", d=d)
                    nc.vector.tensor_add(P3[:, :d - 1, :], E3[:, :d - 1, :], D3[:, 1:, :])
                    nc.vector.tensor_copy(P3[:, d - 1:d, :], E3[:, d - 1:d, :])
                    P[s][c] = pt_

            # ---- per-direction MLP + softmax-weighted sums ----
            pending_numer = None  # (di, Zt, ej) from previous direction

            def emit_numer(di, Zt, ej):
                xre = [xnpool.tile([128, 4096], FP8, tag="xn", name="xre") for _ in range(4)]
                for ci in range(4):
                    nc.sync.dma_start(xre[ci][:], xd[ej][ci * 128:(ci + 1) * 128, :])
                npt = npsum.tile([1, 512], F32, tag="np", name="npt")
                for ci in range(4):
                    for t in range(8):
                        nc.tensor.matmul(
                            npt[:],
                            Zt[:, ci * 8 + t: ci * 8 + t + 1],
                            xre[ci][:, t * 512:(t + 1) * 512],
                            start=(ci == 0 and t == 0),
                            stop=(ci == 3 and t == 7),
                        )
                nst = zspool.tile([1, 512], F32, tag="nst", name="nst")
                nc.vector.tensor_copy(nst[:], npt[:])
                nc.sync.dma_start(onum[di:di + 1, :], nst[:])

            for di, (k, wi, wj, ej) in enumerate(DIRS):
                zt = zpool.tile([1, 4096], BF16, tag="zb", name="zt")
                hbs = [None] * 8  # per-n list of 4 H^T tiles

                def emit_logit(n):
                    lpt = lpsum.tile([1, 512], F32, tag="lp", name="lpt")
                    for m in range(4):
                        nc.tensor.matmul(
                            lpt[:],
                            w2sb[k][:, m:m + 1],
                            hbs[n][m][:],
                            start=(m == 0),
                            stop=(m == 3),
                        )
                    nc.scalar.activation(
                        zt[0:1, n * 512:(n + 1) * 512],
                        lpt[:],
                        Exp,
                        bias=b2sb[0:1, k:k + 1],
                        accum_out=denacc[0:1, di * 8 + n: di * 8 + n + 1],
                    )

                for n in range(8):
                    hbs[n] = []
                    for m in range(4):
                        hpt = hpsum.tile([128, 512], F32, tag="hp", name="hpt")
                        for t in range(4):
                            nc.tensor.matmul(
                                hpt[:],
                                w1sb[k][t][:, m * 128:(m + 1) * 128],
                                P[wi][t][:, n * 512:(n + 1) * 512],
                                start=(t == 0),
                                stop=False,
                            )
                        for t in range(4):
                            nc.tensor.matmul(
                                hpt[:],
                                w1sb[k][4 + t][:, m * 128:(m + 1) * 128],
                                P[wj][t][:, n * 512:(n + 1) * 512],
                                start=False,
                                stop=(t == 3),
                            )
                        hbt = hpool.tile([128, 512], BF16, tag="hb", name="hbt")
                        nc.scalar.activation(hbt[:], hpt[:], Relu, bias=b1sb[k][:, m:m + 1])
                        hbs[n].append(hbt)
                    if n == 2 and pending_numer is not None:
                        emit_numer(*pending_numer)
                        pending_numer = None
                    if n >= 1:
                        emit_logit(n - 1)
                emit_logit(7)

                # z -> partitions (DRAM bounce): Z[p, c*8+t] = z[1024c + 8p + t]
                zdt = zdram_pool.tile([1, 4096], BF16, tag="zd", name="zdt")
                nc.sync.dma_start(zdt[:, :], zt[:, :])
                Zt = zspool.tile([128, 32], BF16, tag="zs", name="Zt")
                nc.sync.dma_start(
                    Zt[:].rearrange("p (c t) -> p c t", c=4),
                    zdt[0, :].rearrange("(c p t) -> p c t", c=4, p=128, t=8),
                )
                pending_numer = (di, Zt, ej)

            emit_numer(*pending_numer)
            pending_numer = None

            # denominators: denacc [1, 6*8] -> den6 [1, 6]
            nc.vector.reduce_sum(
                den6[0:1, :].rearrange("p (k o) -> p k o", o=1),
                denacc[0:1, :].rearrange("p (k n) -> p k n", k=6),
                axis=mybir.AxisListType.X,
            )
            nc.sync.dma_start(oden[:, :], den6[:])

    return nc


_NC = None


def _get_nc():
    global _NC
    if _NC is None:
        _NC = build_nc()
    return _NC


def host_prep(e0, e1, e2, W1, b1, W2, b2):
    """Build the 8 per-core input maps (fp8 X-layout embeds, bf16 weights)."""
    X = [
        np.ascontiguousarray(e).astype(NP_FP8).reshape(B, 512, 4096)
        for e in (e0, e1, e2)
    ]
    w1 = (np.asarray(W1, np.float32) / 27.0).astype(NP_BF16)
    w2 = np.ascontiguousarray(np.asarray(W2, np.float32)[:, :, 0]).reshape(3, 4, 128).astype(NP_BF16)
    b1p = np.ascontiguousarray(np.asarray(b1, np.float32)).reshape(3, 4, 128)
    b2p = np.asarray(b2, np.float32).reshape(1, 3)
    return [
        {
            "x0": X[0][bb], "x1": X[1][bb], "x2": X[2][bb],
            "w1": w1, "w2": w2, "b1": b1p, "b2": b2p,
        }
        for bb in range(B)
    ]


def host_epilogue(results, e0, e1, e2, gamma, beta, Wf, bf):
    tok = (
        e0.astype(np.float32).sum(axis=1)
        + e1.astype(np.float32).sum(axis=1)
        + e2.astype(np.float32).sum(axis=1)
    )  # [B, D]
    matched = np.stack(
        [
            (results[bb]["onum"].astype(np.float32)
             / results[bb]["oden"].astype(np.float32).reshape(6, 1)).sum(axis=0)
            for bb in range(B)
        ]
    )  # [B, D]
    fused = (tok + matched) / float(N_ROWS)
    mu = fused.mean(axis=-1, keepdims=True)
    var = fused.var(axis=-1, keepdims=True)
    ln = (fused - mu) / np.sqrt(var + 1e-5) * gamma + beta
    return (ln @ Wf + bf).astype(np.float32)


def kernel(e0, e1, e2, W1, b1, W2, b2, gamma, beta, Wf, bf):
    nc = _get_nc()
    in_maps = host_prep(e0, e1, e2, W1, b1, W2, b2)
    res = run_bass_kernel_spmd(nc, in_maps, list(range(N_CORES)))
    return host_epilogue(res.results, e0, e1, e2, gamma, beta, Wf, bf)
